# revision 4
# baseline (speedup 1.0000x reference)
"""Trainium2 Bass kernel for nn_MultiHeadHighLevelAllocator.

Math (reference):
    uav_embed = MLP_u(uav_feat)                     # (U=256, E=128)
    task_embed = MLP_t(task_feat)                   # (T=512, E=128)
    uq[h,u,:]  = uav_embed[u] + head_queries[h]     # (H=4, U, E)
    a[hu,k]    = uq[hu] @ Wu.T + fb0                # Wu = fw0[:, :E]
    b[t,k]     = task_embed[t] @ Wt.T               # Wt = fw0[:, E:]
    logits[hu,t] = sum_k fw1[k] * relu(a[hu,k] + b[t,k]) + fb1

Strategy (8 cores, shard T -> 64 t's per core, full HU on every core):
    - Prep matmuls on PE in feature-on-partition layout (host pre-transposes
      inputs), all run as f32r (1 cycle/row at N>=256 vs 4 for plain f32);
      a16[k, hu] evicted to fp16 (2 k-tiles of (128, 1024)); b[k, t_local]
      stays fp32 ((128, 64)/k-tile).
    - Fused bias+ReLU per (t, ktile) unit over the (128k, 1024hu) plane,
      split across THREE engines: DVE tensor_scalar add+max (fp16 4x mode,
      ~0.32us/unit), ACT relu-with-bias (~0.95us/unit), and Pool/gpsimd
      tensor_scalar (~1.5us/unit). Static split CFG[xa]/CFG[xg] over the
      128 units; slow-engine units sit late in PE's consumption order.
    - Contraction with fw1 on PE in fp16 with M=32 zero-padded weight
      columns: round r slot j uses lhsT = fw1z (128, 32) with the fw1
      k-slice in column r, zeros elsewhere, accumulating (start=False)
      into PSUM partition 32j + r. All 64 t's of a core land in ONE
      (128, 1024) psum tile -> a single (x2 halves) ACT eviction (+fb1)
      instead of one per 8 t's, freeing ~13us of ACT time.
    - Output DMA gathers partitions (j,r) -> rows t=4r+j in one strided
      DMA per half.

Output per core: (64, 1024) fp32 [t_local, h*U+u]; host reassembles (H,U,T).
"""

import contextlib

import numpy as np

import concourse.bacc as bacc
import concourse.mybir as mybir
from concourse.tile import TileContext
from concourse.bass_utils import run_bass_kernel_spmd

U, T, H = 256, 512, 4
UAV_DIM, TASK_DIM, E, HID = 64, 32, 128, 256
HU = H * U                      # 1024
NCORES = 8
TL = T // NCORES                # 64 t's per core
NKT = HID // 128                # 2 k-tiles
NROUNDS = TL // 4               # 16 rounds of 4 t's

f32 = mybir.dt.float32
f16 = mybir.dt.float16
f32r = mybir.dt.float32r
AF = mybir.ActivationFunctionType
ALU = mybir.AluOpType
ET = mybir.EngineType

# Tunables; _get_nc caches on their values.
#   xa: total fusion units on ACT; xg: on Pool; rest (128-xa-xg) on DVE
CFG = {"xa": 27, "xg": 19, "rpool": 48}

# (kt, j) units in PE first-touch order (matmul loop is kt-major with
# j order 1,2,3,0); slow engines are assigned from the END of this list.
CONS_ORDER = [(0, 1), (0, 2), (0, 3), (0, 0),
              (1, 1), (1, 2), (1, 3), (1, 0)]


def _assign_units(r):
    na = ((r + 1) * CFG["xa"]) // NROUNDS - (r * CFG["xa"]) // NROUNDS
    ng = ((r + 1) * CFG["xg"]) // NROUNDS - (r * CFG["xg"]) // NROUNDS
    na = min(na, 8)
    ng = min(ng, 8 - na)
    asg = {}
    n = len(CONS_ORDER)
    for i, u in enumerate(CONS_ORDER):
        if i >= n - ng:
            asg[u] = "pool"
        elif i >= n - ng - na:
            asg[u] = "act"
        else:
            asg[u] = "dve"
    return asg


IN_SPECS = [
    ("uavT", (UAV_DIM, U), f32r),
    ("uw0T", (UAV_DIM, 128), f32r),
    ("uw1T", (128, 128), f32r),
    ("uw2T", (128, E), f32r),
    ("ub0c", (128, 1), f32),
    ("ub1c", (128, 1), f32),
    ("hq2T", (E, H), f32),
    ("taskT", (TASK_DIM, TL), f32r),
    ("tw0T", (TASK_DIM, 128), f32r),
    ("tw1T", (128, 128), f32r),
    ("tw2T", (128, E), f32r),
    ("tb0c", (128, 1), f32),
    ("tb1c", (128, 1), f32),
    ("tb2c", (128, 1), f32),
    ("WuT", (E, HID), f32r),
    ("WtT", (E, HID), f32r),
    ("fb0c", (128, NKT), f32),
    ("fw1z", (128, NKT * NROUNDS * 32), f16),
    ("fb1s", (128, 1), f32),
]


def _emit_loads(nc, d, singles):
    s = {}
    for name, shape, dt_ in IN_SPECS:
        s[name] = singles.tile(list(shape), dt_, name=name, tag=name)
        nc.sync.dma_start(out=s[name], in_=d[name][:])
    return s


def _mm(nc, out, lhsT, rhs, **kw):
    nc.tensor.matmul(out, lhsT, rhs, start=True, stop=True, **kw)


def _emit_body(nc, d, s, pools, it=0):
    singles, prep, ppsum, rpool, opool, fpsum = pools

    # ---- encoders + a/b prep ----
    uqT_s = singles.tile([E, HU], f32r, name="uqT", tag="uqT")
    a16_s = [singles.tile([128, HU], f16, tag=f"a16_{kt}", name=f"a16_{kt}")
             for kt in range(NKT)]
    b_s = [singles.tile([128, TL], f32, tag=f"b{kt}", name=f"b{kt}")
           for kt in range(NKT)]

    # uav + task encoders, chains interleaved so PE/ACT ping-pong.
    pe1 = ppsum.tile([128, U], f32, tag="ps_o", name="pe1")
    _mm(nc, pe1, s["uw0T"], s["uavT"])
    pt1 = ppsum.tile([128, TL], f32, tag="ps_o", name="pt1")
    _mm(nc, pt1, s["tw0T"], s["taskT"])
    h1 = prep.tile([128, U], f32r, tag="pr", name="h1")
    nc.scalar.activation(h1, pe1, AF.Relu, bias=s["ub0c"][:, 0:1])
    s1 = prep.tile([128, TL], f32r, tag="pr", name="s1")
    nc.scalar.activation(s1, pt1, AF.Relu, bias=s["tb0c"][:, 0:1])
    pe2 = ppsum.tile([128, U], f32, tag="ps_o", name="pe2")
    _mm(nc, pe2, s["uw1T"], h1)
    pt2 = ppsum.tile([128, TL], f32, tag="ps_o", name="pt2")
    _mm(nc, pt2, s["tw1T"], s1)
    h2 = prep.tile([128, U], f32r, tag="pr", name="h2")
    nc.scalar.activation(h2, pe2, AF.Relu, bias=s["ub1c"][:, 0:1])
    s2 = prep.tile([128, TL], f32r, tag="pr", name="s2")
    nc.scalar.activation(s2, pt2, AF.Relu, bias=s["tb1c"][:, 0:1])
    pe3 = ppsum.tile([E, U], f32, tag="ps_o", name="pe3")
    _mm(nc, pe3, s["uw2T"], h2)
    pt3 = ppsum.tile([E, TL], f32, tag="ps_o", name="pt3")
    _mm(nc, pt3, s["tw2T"], s2)
    # uqT[:, h-block] = uav_embedT + (head_queries[h] + ub2)   (ACT)
    for h in range(H):
        nc.scalar.activation(
            uqT_s[:, h * U : (h + 1) * U], pe3, AF.Identity,
            bias=s["hq2T"][:, h : h + 1],
        )
    teT = prep.tile([E, TL], f32r, tag="pr", name="teT")
    nc.scalar.activation(teT, pt3, AF.Identity, bias=s["tb2c"][:, 0:1])

    # b[kt] = (WtT slice).T @ teT  -> (128, TL)
    for kt in range(NKT):
        pb = ppsum.tile([128, TL], f32, tag="ps_o", name=f"pb{kt}")
        _mm(nc, pb, s["WtT"][:, kt * 128 : (kt + 1) * 128], teT)
        nc.vector.tensor_copy(out=b_s[kt], in_=pb)

    # a[kt] = (WuT slice).T @ uqT + fb0  -> (128, HU) fp16
    for kt in range(NKT):
        for half in range(2):
            pa = ppsum.tile([128, 512], f32, tag="ps_o", name=f"pa{kt}{half}")
            _mm(nc, pa, s["WuT"][:, kt * 128 : (kt + 1) * 128],
                uqT_s[:, half * 512 : (half + 1) * 512])
            nc.scalar.activation(
                a16_s[kt][:, half * 512 : (half + 1) * 512], pa,
                AF.Identity, bias=s["fb0c"][:, kt : kt + 1],
            )

    # ---- fusion: 16 rounds of 4 t's, one psum tile for all 64 t's ----
    ps_all = fpsum.tile([128, HU], f32, tag="ps_f", name=f"ps_all{it}")
    for r in range(NROUNDS):
        asg = _assign_units(r)
        rt = {}
        for (kt, j) in CONS_ORDER:
            t = 4 * r + j
            Rt = rpool.tile([128, HU], f16, tag="R",
                            name=f"R{it}_{r}_{j}_{kt}")
            bias_ap = b_s[kt][:, t : t + 1]
            eng = asg[(kt, j)]
            if eng == "act":
                nc.scalar.activation(Rt, a16_s[kt], AF.Relu, bias=bias_ap)
            elif eng == "pool":
                nc.gpsimd.tensor_scalar(
                    out=Rt, in0=a16_s[kt], scalar1=bias_ap,
                    scalar2=0.0, op0=ALU.add, op1=ALU.max,
                )
            else:
                nc.vector.tensor_scalar(
                    out=Rt, in0=a16_s[kt], scalar1=bias_ap,
                    scalar2=0.0, op0=ALU.add, op1=ALU.max,
                )
            rt[(kt, j)] = Rt
        for kt in range(NKT):
            for half in range(2):
                for j in (1, 2, 3, 0):
                    nc.tensor.matmul(
                        ps_all[32 * j : 32 * j + 32,
                               half * 512 : (half + 1) * 512],
                        s["fw1z"][:, (kt * NROUNDS + r) * 32 :
                                  (kt * NROUNDS + r) * 32 + 32],
                        rt[(kt, j)][:, half * 512 : (half + 1) * 512],
                        start=(r == 0 and kt == 0),
                        stop=(r == NROUNDS - 1 and kt == NKT - 1),
                        tile_position=(0, 32 * j),
                        skip_group_check=True,
                    )

    # ---- single eviction (+fb1), then one gather DMA per quad ----
    # psum partition 32j + r holds t = 4r + j; each DMA j covers rows
    # t = j, j+4, ..., j+60 (dram stride 4) from partitions 32j..32j+15.
    o_st = opool.tile([128, HU], f32, tag="o", name=f"o{it}")
    odst = d["out"].rearrange("(r j) n -> r j n", j=4)
    for half in range(2):
        cs = slice(half * 512, (half + 1) * 512)
        nc.scalar.activation(o_st[:, cs], ps_all[:, cs], AF.Identity,
                             bias=s["fb1s"][:, 0:1])
        for j in range(4):
            nc.sync.dma_start(out=odst[:, j, cs],
                              in_=o_st[32 * j : 32 * j + NROUNDS, cs])


def _build_nc(mult=1, loop=None):
    nc = bacc.Bacc(None, target_bir_lowering=False)
    d = {}
    for name, shape, dt_ in IN_SPECS:
        d[name] = nc.dram_tensor(name, list(shape), dt_, kind="ExternalInput")
    d["out"] = nc.dram_tensor("out", [TL, HU], f32, kind="ExternalOutput")

    with TileContext(nc) as tc:
        with tc.tile_pool(name="singles", bufs=1) as singles, \
             tc.tile_pool(name="prep", bufs=2) as prep, \
             tc.tile_pool(name="rpool", bufs=CFG["rpool"]) as rpool, \
             tc.tile_pool(name="opool", bufs=2) as opool, \
             tc.tile_pool(name="fpsum", bufs=2, space="PSUM") as fpsum:
            pools = (singles, prep, fpsum, rpool, opool, fpsum)
            s = _emit_loads(nc, d, singles)
            ctx = (tc.For_i(0, loop, 1,
                            hint_engines=(ET.PE, ET.Activation, ET.DVE,
                                          ET.Pool))
                   if loop else contextlib.nullcontext())
            with ctx:
                for it in range(mult):
                    _emit_body(nc, d, s, pools, it)

    nc.finalize()
    return nc


_NC_CACHE = {}


def _get_nc(mult=1, loop=None):
    key = (mult, loop, tuple(sorted(CFG.items())))
    if key not in _NC_CACHE:
        _NC_CACHE[key] = _build_nc(mult, loop)
    return _NC_CACHE[key]


def _prep_inputs(inputs):
    ct = np.ascontiguousarray
    f = np.float32
    uav_feat = inputs["uav_feat"].astype(f)
    task_feat = inputs["task_feat"].astype(f)
    fw1_kt = inputs["fw1"].reshape(NKT, 128).astype(np.float16)
    fw1z = np.zeros((128, NKT * NROUNDS * 32), dtype=np.float16)
    for kt in range(NKT):
        for r in range(NROUNDS):
            fw1z[:, (kt * NROUNDS + r) * 32 + r] = fw1_kt[kt]
    base = {
        "uavT": ct(uav_feat.T),
        "uw0T": ct(inputs["uw0"].T.astype(f)),
        "uw1T": ct(inputs["uw1"].T.astype(f)),
        "uw2T": ct(inputs["uw2"].T.astype(f)),
        "ub0c": ct(inputs["ub0"].astype(f).reshape(128, 1)),
        "ub1c": ct(inputs["ub1"].astype(f).reshape(128, 1)),
        "hq2T": ct((inputs["head_queries"].astype(f)
                    + inputs["ub2"].astype(f)[None, :]).T),
        "tw0T": ct(inputs["tw0"].T.astype(f)),
        "tw1T": ct(inputs["tw1"].T.astype(f)),
        "tw2T": ct(inputs["tw2"].T.astype(f)),
        "tb0c": ct(inputs["tb0"].astype(f).reshape(128, 1)),
        "tb1c": ct(inputs["tb1"].astype(f).reshape(128, 1)),
        "tb2c": ct(inputs["tb2"].astype(f).reshape(128, 1)),
        "WuT": ct(inputs["fw0"][:, :E].T.astype(f)),
        "WtT": ct(inputs["fw0"][:, E:].T.astype(f)),
        "fb0c": ct(inputs["fb0"].astype(f).reshape(NKT, 128).T),
        "fw1z": fw1z,
        "fb1s": ct(np.full((128, 1), float(inputs["fb1"][0]), dtype=f)),
    }
    taskT_full = ct(task_feat.T)
    in_maps = []
    for c in range(NCORES):
        m = dict(base)
        m["taskT"] = ct(taskT_full[:, c * TL : (c + 1) * TL])
        in_maps.append(m)
    return in_maps


def run(trace=False, **inputs):
    nc = _get_nc()
    in_maps = _prep_inputs(inputs)
    res = run_bass_kernel_spmd(nc, in_maps, list(range(NCORES)), trace=trace)
    big = np.concatenate([res.results[c]["out"] for c in range(NCORES)], axis=0)
    out = np.ascontiguousarray(big.T).reshape(H, U, T)
    return out, res


def kernel(**inputs):
    out, _ = run(**inputs)
    return out


# revision 10
# speedup vs baseline: 6.5241x; 6.5241x over previous
"""Trainium2 Bass kernel for nn_MultiHeadHighLevelAllocator.

Math (reference):
    uav_embed = MLP_u(uav_feat)                     # (U=256, E=128)
    task_embed = MLP_t(task_feat)                   # (T=512, E=128)
    uq[h,u,:]  = uav_embed[u] + head_queries[h]     # (H=4, U, E)
    a[hu,k]    = uq[hu] @ Wu.T + fb0                # Wu = fw0[:, :E]
    b[t,k]     = task_embed[t] @ Wt.T               # Wt = fw0[:, E:]
    logits[hu,t] = sum_k fw1[k] * relu(a[hu,k] + b[t,k]) + fb1

Strategy (8 cores, shard T -> 64 t's per core, full HU on every core):
    - Prep matmuls on PE in feature-on-partition layout (host pre-transposes
      inputs), all f32r (1 cycle/row at N>=256 vs 4 for plain f32; tensors
      typed f32r end-to-end so the BIR verifier sees rounded producers);
      a16[k, hu] in fp16 (2 k-tiles of (128, 1024)); b[k, t_local] f32.
    - Fused bias+ReLU per (t, ktile) unit over the (128k, 1024hu) plane,
      split DVE (tensor_scalar add+max fp16, ~0.46us/unit measured) vs ACT
      (relu-with-bias, ~1.24us/unit measured): CFG[xa]=37 units on ACT,
      rest DVE; ACT units sit late in PE's consumption order. gpsimd
      tensor_scalar measured ~15us/unit on HW (vs 1.5us in the cost
      model) - Pool is disabled (CFG[xg]=0).
    - Contraction with fw1 on PE in fp16 with M=16 zero-padded weight
      columns: round r slot j uses lhsT = fw1z (128, 16) with the fw1
      k-slice in column r, zeros elsewhere, accumulating (start=False)
      into PSUM partition 32j + r. All 64 t's of a core land in ONE
      (128, 1024) psum tile -> a single (x2 halves) eviction instead of
      one per 8 t's (saved ~13us of ACT time vs the old scheme).
    - Software pipelining (CFG[pp]): loop body holds TWO ping-pong
      iterations; prep for the next iteration is emitted one step per
      fusion round (CFG[ppk]=1) so its serial PE->ACT chain hides under
      fusion. Prep identity-type evictions (uqT/teT/a16) run on DVE
      (CFG[pid]=1) and the final psum eviction too (CFG[pev]=1), keeping
      ACT a pure-Relu stream (avoids act-table churn; measured -4us).
    - Output DMA per quad j gathers partitions 32j..32j+15 -> dram rows
      t = j, j+4, ..., j+60 (stride 4).

Measured (loop-slope, n1=1024/n2=32768): 67.7us baseline -> ~49us.
Output per core: (64, 1024) fp32 [t_local, h*U+u]; host reassembles (H,U,T).
"""

import contextlib

import numpy as np

import concourse.bacc as bacc
import concourse.mybir as mybir
from concourse.tile import TileContext
from concourse.bass_utils import run_bass_kernel_spmd

U, T, H = 256, 512, 4
UAV_DIM, TASK_DIM, E, HID = 64, 32, 128, 256
HU = H * U                      # 1024
NCORES = 8
TL = T // NCORES                # 64 t's per core
NKT = HID // 128                # 2 k-tiles
NROUNDS = TL // 4               # 16 rounds of 4 t's

f32 = mybir.dt.float32
f16 = mybir.dt.float16
f32r = mybir.dt.float32r
AF = mybir.ActivationFunctionType
ALU = mybir.AluOpType
ET = mybir.EngineType

# Tunables; _get_nc caches on their values.
#   xa: total fusion units on ACT; xg: on Pool; rest (128-xa-xg) on DVE
CFG = {"xa": 37, "xg": 0, "rpool": 48, "mw": 16, "pp": 1, "ppk": 1, "pid": 1, "pev": 1, "xaw": 16}

# (kt, j) units in PE first-touch order (matmul loop is kt-major with
# j order 1,2,3,0); slow engines are assigned from the END of this list.
CONS_ORDER = [(0, 1), (0, 2), (0, 3), (0, 0),
              (1, 1), (1, 2), (1, 3), (1, 0)]


def _assign_units(r):
    w = CFG["xaw"]
    if r < w:
        na = ((r + 1) * CFG["xa"]) // w - (r * CFG["xa"]) // w
    else:
        na = 0
    ng = ((r + 1) * CFG["xg"]) // NROUNDS - (r * CFG["xg"]) // NROUNDS
    na = min(na, 8)
    ng = min(ng, 8 - na)
    asg = {}
    n = len(CONS_ORDER)
    for i, u in enumerate(CONS_ORDER):
        if i >= n - ng:
            asg[u] = "pool"
        elif i >= n - ng - na:
            asg[u] = "act"
        else:
            asg[u] = "dve"
    return asg


IN_SPECS = [
    ("uavT", (UAV_DIM, U), f32r),
    ("uw0T", (UAV_DIM, 128), f32r),
    ("uw1T", (128, 128), f32r),
    ("uw2T", (128, E), f32r),
    ("ub0c", (128, 1), f32),
    ("ub1c", (128, 1), f32),
    ("hq2T", (E, H), f32),
    ("taskT", (TASK_DIM, TL), f32r),
    ("tw0T", (TASK_DIM, 128), f32r),
    ("tw1T", (128, 128), f32r),
    ("tw2T", (128, E), f32r),
    ("tb0c", (128, 1), f32),
    ("tb1c", (128, 1), f32),
    ("tb2c", (128, 1), f32),
    ("WuT", (E, HID), f32r),
    ("WtT", (E, HID), f32r),
    ("fb0c", (128, NKT), f32),
    ("fw1z", (128, NKT * NROUNDS * 32), f16),  # mw<=32 cols per slot
    ("fb1s", (128, 1), f32),
]


def _emit_loads(nc, d, singles):
    s = {}
    for name, shape, dt_ in IN_SPECS:
        s[name] = singles.tile(list(shape), dt_, name=name, tag=name)
        nc.sync.dma_start(out=s[name], in_=d[name][:])
    return s


def _mm(nc, out, lhsT, rhs, **kw):
    nc.tensor.matmul(out, lhsT, rhs, start=True, stop=True, **kw)


def _make_state(spool, p):
    st = {}
    st["uqT"] = spool.tile([E, HU], f32r, name=f"uqT{p}", tag=f"uqT{p}")
    st["a16"] = [spool.tile([128, HU], f16, tag=f"a16_{kt}_{p}",
                            name=f"a16_{kt}_{p}") for kt in range(NKT)]
    st["b"] = [spool.tile([128, TL], f32, tag=f"b{kt}_{p}",
                          name=f"b{kt}_{p}") for kt in range(NKT)]
    return st


def _emit_prep(nc, s, pools, st):
    """Generator of prep steps (encoders -> uqT -> a16/b) writing into
    state tiles `st`. Each yield is an interleave point so the fusion
    loop can spread the serial chain across its rounds."""
    singles, prep, ppsum, rpool, opool, fpsum = pools
    uqT_s, a16_s, b_s = st["uqT"], st["a16"], st["b"]

    pe1 = ppsum.tile([128, U], f32, tag="ps_o", name="pe1")
    _mm(nc, pe1, s["uw0T"], s["uavT"])
    pt1 = ppsum.tile([128, TL], f32, tag="ps_o", name="pt1")
    _mm(nc, pt1, s["tw0T"], s["taskT"])
    yield
    h1 = prep.tile([128, U], f32r, tag="pr", name="h1")
    nc.scalar.activation(h1, pe1, AF.Relu, bias=s["ub0c"][:, 0:1])
    s1 = prep.tile([128, TL], f32r, tag="pr", name="s1")
    nc.scalar.activation(s1, pt1, AF.Relu, bias=s["tb0c"][:, 0:1])
    yield
    pe2 = ppsum.tile([128, U], f32, tag="ps_o", name="pe2")
    _mm(nc, pe2, s["uw1T"], h1)
    pt2 = ppsum.tile([128, TL], f32, tag="ps_o", name="pt2")
    _mm(nc, pt2, s["tw1T"], s1)
    yield
    h2 = prep.tile([128, U], f32r, tag="pr", name="h2")
    nc.scalar.activation(h2, pe2, AF.Relu, bias=s["ub1c"][:, 0:1])
    s2 = prep.tile([128, TL], f32r, tag="pr", name="s2")
    nc.scalar.activation(s2, pt2, AF.Relu, bias=s["tb1c"][:, 0:1])
    yield
    pe3 = ppsum.tile([E, U], f32, tag="ps_o", name="pe3")
    _mm(nc, pe3, s["uw2T"], h2)
    pt3 = ppsum.tile([E, TL], f32, tag="ps_o", name="pt3")
    _mm(nc, pt3, s["tw2T"], s2)
    yield
    def _ident(out, in_, bias_ap):
        if CFG["pid"]:
            nc.vector.tensor_scalar_add(out, in_, bias_ap)
        else:
            nc.scalar.activation(out, in_, AF.Identity, bias=bias_ap)

    for h in range(2):
        _ident(uqT_s[:, h * U : (h + 1) * U], pe3, s["hq2T"][:, h : h + 1])
    yield
    for h in range(2, 4):
        _ident(uqT_s[:, h * U : (h + 1) * U], pe3, s["hq2T"][:, h : h + 1])
    teT = prep.tile([E, TL], f32r, tag="pr", name="teT")
    _ident(teT, pt3, s["tb2c"][:, 0:1])
    yield
    for kt in range(NKT):
        pb = ppsum.tile([128, TL], f32, tag="ps_o", name=f"pb{kt}")
        _mm(nc, pb, s["WtT"][:, kt * 128 : (kt + 1) * 128], teT)
        nc.vector.tensor_copy(out=b_s[kt], in_=pb)
        yield
    for kt in range(NKT):
        for half in range(2):
            pa = ppsum.tile([128, 512], f32, tag="ps_o", name=f"pa{kt}{half}")
            _mm(nc, pa, s["WuT"][:, kt * 128 : (kt + 1) * 128],
                uqT_s[:, half * 512 : (half + 1) * 512])
            _ident(a16_s[kt][:, half * 512 : (half + 1) * 512], pa,
                   s["fb0c"][:, kt : kt + 1])
            yield


def _emit_fusion(nc, d, s, pools, st, prep_gen, it=0):
    singles, prep, ppsum, rpool, opool, fpsum = pools
    a16_s, b_s = st["a16"], st["b"]

    ps_all = fpsum.tile([128, HU], f32, tag="ps_f", name=f"ps_all{it}")
    for r in range(NROUNDS):
        asg = _assign_units(r)
        rt = {}
        for (kt, j) in CONS_ORDER:
            t = 4 * r + j
            Rt = rpool.tile([128, HU], f16, tag="R",
                            name=f"R{it}_{r}_{j}_{kt}")
            bias_ap = b_s[kt][:, t : t + 1]
            eng = asg[(kt, j)]
            if eng == "act":
                nc.scalar.activation(Rt, a16_s[kt], AF.Relu, bias=bias_ap)
            elif eng == "pool":
                nc.gpsimd.tensor_scalar(
                    out=Rt, in0=a16_s[kt], scalar1=bias_ap,
                    scalar2=0.0, op0=ALU.add, op1=ALU.max,
                )
            else:
                nc.vector.tensor_scalar(
                    out=Rt, in0=a16_s[kt], scalar1=bias_ap,
                    scalar2=0.0, op0=ALU.add, op1=ALU.max,
                )
            rt[(kt, j)] = Rt
        for _ in range(CFG["ppk"]):
            next(prep_gen, None)
        mw = CFG["mw"]
        for kt in range(NKT):
            for half in range(2):
                for j in (1, 2, 3, 0):
                    nc.tensor.matmul(
                        ps_all[32 * j : 32 * j + mw,
                               half * 512 : (half + 1) * 512],
                        s["fw1z"][:, (kt * NROUNDS + r) * 32 :
                                  (kt * NROUNDS + r) * 32 + mw],
                        rt[(kt, j)][:, half * 512 : (half + 1) * 512],
                        start=(r == 0 and kt == 0),
                        stop=(r == NROUNDS - 1 and kt == NKT - 1),
                        tile_position=(0, 32 * j),
                        skip_group_check=True,
                    )
    for _ in prep_gen:
        pass

    # ---- single eviction (+fb1), then one gather DMA per quad ----
    # psum partition 32j + r holds t = 4r + j; each DMA j covers rows
    # t = j, j+4, ..., j+60 (dram stride 4) from partitions 32j..32j+15.
    o_st = opool.tile([128, HU], f32, tag="o", name=f"o{it}")
    odst = d["out"].rearrange("(r j) n -> r j n", j=4)
    for half in range(2):
        cs = slice(half * 512, (half + 1) * 512)
        if CFG["pev"]:
            nc.vector.tensor_scalar_add(o_st[:, cs], ps_all[:, cs],
                                        s["fb1s"][:, 0:1])
        else:
            nc.scalar.activation(o_st[:, cs], ps_all[:, cs], AF.Identity,
                                 bias=s["fb1s"][:, 0:1])
        for j in range(4):
            nc.sync.dma_start(out=odst[:, j, cs],
                              in_=o_st[32 * j : 32 * j + NROUNDS, cs])


def _build_nc(mult=1, loop=None):
    nc = bacc.Bacc(None, target_bir_lowering=False)
    d = {}
    for name, shape, dt_ in IN_SPECS:
        d[name] = nc.dram_tensor(name, list(shape), dt_, kind="ExternalInput")
    d["out"] = nc.dram_tensor("out", [TL, HU], f32, kind="ExternalOutput")

    pp = CFG["pp"] and loop is not None
    with TileContext(nc) as tc:
        with tc.tile_pool(name="singles", bufs=1) as singles, \
             tc.tile_pool(name="prep", bufs=2) as prep, \
             tc.tile_pool(name="rpool", bufs=CFG["rpool"]) as rpool, \
             tc.tile_pool(name="opool", bufs=2) as opool, \
             tc.tile_pool(name="fpsum", bufs=2, space="PSUM") as fpsum:
            pools = (singles, prep, fpsum, rpool, opool, fpsum)
            s = _emit_loads(nc, d, singles)
            hints = (ET.PE, ET.Activation, ET.DVE) + (
                (ET.Pool,) if CFG["xg"] else ())
            if pp:
                st = [_make_state(singles, p) for p in range(2)]
                for _ in _emit_prep(nc, s, pools, st[0]):
                    pass
                with tc.For_i(0, loop // 2, 1, hint_engines=hints):
                    for p in range(2):
                        _emit_fusion(nc, d, s, pools, st[p],
                                     _emit_prep(nc, s, pools, st[1 - p]),
                                     it=p)
            else:
                st = _make_state(singles, 0)
                ctx = (tc.For_i(0, loop, 1, hint_engines=hints)
                       if loop else contextlib.nullcontext())
                with ctx:
                    for it in range(mult):
                        for _ in _emit_prep(nc, s, pools, st):
                            pass
                        _emit_fusion(nc, d, s, pools, st, iter(()), it=it)

    nc.finalize()
    return nc


_NC_CACHE = {}


def _get_nc(mult=1, loop=None):
    key = (mult, loop, tuple(sorted(CFG.items())))
    if key not in _NC_CACHE:
        _NC_CACHE[key] = _build_nc(mult, loop)
    return _NC_CACHE[key]


def _prep_inputs(inputs):
    ct = np.ascontiguousarray
    f = np.float32
    uav_feat = inputs["uav_feat"].astype(f)
    task_feat = inputs["task_feat"].astype(f)
    fw1_kt = inputs["fw1"].reshape(NKT, 128).astype(np.float16)
    fw1z = np.zeros((128, NKT * NROUNDS * 32), dtype=np.float16)
    for kt in range(NKT):
        for r in range(NROUNDS):
            fw1z[:, (kt * NROUNDS + r) * 32 + r] = fw1_kt[kt]
    base = {
        "uavT": ct(uav_feat.T),
        "uw0T": ct(inputs["uw0"].T.astype(f)),
        "uw1T": ct(inputs["uw1"].T.astype(f)),
        "uw2T": ct(inputs["uw2"].T.astype(f)),
        "ub0c": ct(inputs["ub0"].astype(f).reshape(128, 1)),
        "ub1c": ct(inputs["ub1"].astype(f).reshape(128, 1)),
        "hq2T": ct((inputs["head_queries"].astype(f)
                    + inputs["ub2"].astype(f)[None, :]).T),
        "tw0T": ct(inputs["tw0"].T.astype(f)),
        "tw1T": ct(inputs["tw1"].T.astype(f)),
        "tw2T": ct(inputs["tw2"].T.astype(f)),
        "tb0c": ct(inputs["tb0"].astype(f).reshape(128, 1)),
        "tb1c": ct(inputs["tb1"].astype(f).reshape(128, 1)),
        "tb2c": ct(inputs["tb2"].astype(f).reshape(128, 1)),
        "WuT": ct(inputs["fw0"][:, :E].T.astype(f)),
        "WtT": ct(inputs["fw0"][:, E:].T.astype(f)),
        "fb0c": ct(inputs["fb0"].astype(f).reshape(NKT, 128).T),
        "fw1z": fw1z,
        "fb1s": ct(np.full((128, 1), float(inputs["fb1"][0]), dtype=f)),
    }
    taskT_full = ct(task_feat.T)
    in_maps = []
    for c in range(NCORES):
        m = dict(base)
        m["taskT"] = ct(taskT_full[:, c * TL : (c + 1) * TL])
        in_maps.append(m)
    return in_maps


def run(trace=False, **inputs):
    nc = _get_nc()
    in_maps = _prep_inputs(inputs)
    res = run_bass_kernel_spmd(nc, in_maps, list(range(NCORES)), trace=trace)
    big = np.concatenate([res.results[c]["out"] for c in range(NCORES)], axis=0)
    out = np.ascontiguousarray(big.T).reshape(H, U, T)
    return out, res


def kernel(**inputs):
    out, _ = run(**inputs)
    return out


# revision 11
# speedup vs baseline: 6.7274x; 1.0312x over previous
"""Trainium2 Bass kernel for nn_MultiHeadHighLevelAllocator.

Math (reference):
    uav_embed = MLP_u(uav_feat)                     # (U=256, E=128)
    task_embed = MLP_t(task_feat)                   # (T=512, E=128)
    uq[h,u,:]  = uav_embed[u] + head_queries[h]     # (H=4, U, E)
    a[hu,k]    = uq[hu] @ Wu.T + fb0                # Wu = fw0[:, :E]
    b[t,k]     = task_embed[t] @ Wt.T               # Wt = fw0[:, E:]
    logits[hu,t] = sum_k fw1[k] * relu(a[hu,k] + b[t,k]) + fb1

Strategy (8 cores, shard T -> 64 t's per core, full HU on every core):
    - Prep matmuls on PE in feature-on-partition layout (host pre-transposes
      inputs), all f32r (1 cycle/row at N>=256 vs 4 for plain f32; tensors
      typed f32r end-to-end so the BIR verifier sees rounded producers);
      a16[k, hu] in fp16 (2 k-tiles of (128, 1024)); b[k, t_local] f32.
    - Fused bias+ReLU per (t, ktile) unit over the (128k, 1024hu) plane,
      split DVE (tensor_scalar add+max fp16, ~0.46us/unit measured) vs ACT
      (relu-with-bias, ~1.24us/unit measured): CFG[xa]=37 units on ACT,
      rest DVE; ACT units sit late in PE's consumption order. gpsimd
      tensor_scalar measured ~15us/unit on HW (vs 1.5us in the cost
      model) - Pool is disabled (CFG[xg]=0).
    - Contraction with fw1 on PE in fp16 with M=16 zero-padded weight
      columns: round r slot j uses lhsT = fw1z (128, 16) with the fw1
      k-slice in column r, zeros elsewhere, accumulating (start=False)
      into PSUM partition 32j + r. All 64 t's of a core land in ONE
      (128, 1024) psum tile -> a single (x2 halves) eviction instead of
      one per 8 t's (saved ~13us of ACT time vs the old scheme).
    - Software pipelining (CFG[pp]): loop body holds TWO ping-pong
      iterations; prep for the next iteration is emitted one step per
      fusion round (CFG[ppk]=2) so its serial PE->ACT chain hides under
      fusion. Prep identity-type evictions (uqT/teT/a16) run on DVE
      (CFG[pid]=1) and the final psum eviction too (CFG[pev]=1), keeping
      ACT a pure-Relu stream (avoids act-table churn; measured -4us).
    - Output DMA per quad j gathers partitions 32j..32j+15 -> dram rows
      t = j, j+4, ..., j+60 (stride 4).

Measured (loop-slope, n1=1024/n2=32768): 67.7us baseline -> ~47-49us.
Output per core: (64, 1024) fp32 [t_local, h*U+u]; host reassembles (H,U,T).
"""

import contextlib

import numpy as np

import concourse.bacc as bacc
import concourse.mybir as mybir
from concourse.tile import TileContext
from concourse.bass_utils import run_bass_kernel_spmd

U, T, H = 256, 512, 4
UAV_DIM, TASK_DIM, E, HID = 64, 32, 128, 256
HU = H * U                      # 1024
NCORES = 8
TL = T // NCORES                # 64 t's per core
NKT = HID // 128                # 2 k-tiles
NROUNDS = TL // 4               # 16 rounds of 4 t's

f32 = mybir.dt.float32
f16 = mybir.dt.float16
f32r = mybir.dt.float32r
AF = mybir.ActivationFunctionType
ALU = mybir.AluOpType
ET = mybir.EngineType

# Tunables; _get_nc caches on their values.
#   xa: total fusion units on ACT; xg: on Pool; rest (128-xa-xg) on DVE
CFG = {"xa": 37, "xg": 0, "rpool": 48, "mw": 16, "pp": 1, "ppk": 2, "pid": 1, "pev": 1, "xaw": 16}

# (kt, j) units in PE first-touch order (matmul loop is kt-major with
# j order 1,2,3,0); slow engines are assigned from the END of this list.
CONS_ORDER = [(0, 1), (0, 2), (0, 3), (0, 0),
              (1, 1), (1, 2), (1, 3), (1, 0)]


def _assign_units(r):
    w = CFG["xaw"]
    if r < w:
        na = ((r + 1) * CFG["xa"]) // w - (r * CFG["xa"]) // w
    else:
        na = 0
    ng = ((r + 1) * CFG["xg"]) // NROUNDS - (r * CFG["xg"]) // NROUNDS
    na = min(na, 8)
    ng = min(ng, 8 - na)
    asg = {}
    n = len(CONS_ORDER)
    for i, u in enumerate(CONS_ORDER):
        if i >= n - ng:
            asg[u] = "pool"
        elif i >= n - ng - na:
            asg[u] = "act"
        else:
            asg[u] = "dve"
    return asg


IN_SPECS = [
    ("uavT", (UAV_DIM, U), f32r),
    ("uw0T", (UAV_DIM, 128), f32r),
    ("uw1T", (128, 128), f32r),
    ("uw2T", (128, E), f32r),
    ("ub0c", (128, 1), f32),
    ("ub1c", (128, 1), f32),
    ("hq2T", (E, H), f32),
    ("taskT", (TASK_DIM, TL), f32r),
    ("tw0T", (TASK_DIM, 128), f32r),
    ("tw1T", (128, 128), f32r),
    ("tw2T", (128, E), f32r),
    ("tb0c", (128, 1), f32),
    ("tb1c", (128, 1), f32),
    ("tb2c", (128, 1), f32),
    ("WuT", (E, HID), f32r),
    ("WtT", (E, HID), f32r),
    ("fb0c", (128, NKT), f32),
    ("fw1z", (128, NKT * NROUNDS * 32), f16),  # mw<=32 cols per slot
    ("fb1s", (128, 1), f32),
]


def _emit_loads(nc, d, singles):
    s = {}
    for name, shape, dt_ in IN_SPECS:
        s[name] = singles.tile(list(shape), dt_, name=name, tag=name)
        nc.sync.dma_start(out=s[name], in_=d[name][:])
    return s


def _mm(nc, out, lhsT, rhs, **kw):
    nc.tensor.matmul(out, lhsT, rhs, start=True, stop=True, **kw)


def _make_state(spool, p):
    st = {}
    st["uqT"] = spool.tile([E, HU], f32r, name=f"uqT{p}", tag=f"uqT{p}")
    st["a16"] = [spool.tile([128, HU], f16, tag=f"a16_{kt}_{p}",
                            name=f"a16_{kt}_{p}") for kt in range(NKT)]
    st["b"] = [spool.tile([128, TL], f32, tag=f"b{kt}_{p}",
                          name=f"b{kt}_{p}") for kt in range(NKT)]
    return st


def _emit_prep(nc, s, pools, st):
    """Generator of prep steps (encoders -> uqT -> a16/b) writing into
    state tiles `st`. Each yield is an interleave point so the fusion
    loop can spread the serial chain across its rounds."""
    singles, prep, ppsum, rpool, opool, fpsum = pools
    uqT_s, a16_s, b_s = st["uqT"], st["a16"], st["b"]

    pe1 = ppsum.tile([128, U], f32, tag="ps_o", name="pe1")
    _mm(nc, pe1, s["uw0T"], s["uavT"])
    pt1 = ppsum.tile([128, TL], f32, tag="ps_o", name="pt1")
    _mm(nc, pt1, s["tw0T"], s["taskT"])
    yield
    h1 = prep.tile([128, U], f32r, tag="pr", name="h1")
    nc.scalar.activation(h1, pe1, AF.Relu, bias=s["ub0c"][:, 0:1])
    s1 = prep.tile([128, TL], f32r, tag="pr", name="s1")
    nc.scalar.activation(s1, pt1, AF.Relu, bias=s["tb0c"][:, 0:1])
    yield
    pe2 = ppsum.tile([128, U], f32, tag="ps_o", name="pe2")
    _mm(nc, pe2, s["uw1T"], h1)
    pt2 = ppsum.tile([128, TL], f32, tag="ps_o", name="pt2")
    _mm(nc, pt2, s["tw1T"], s1)
    yield
    h2 = prep.tile([128, U], f32r, tag="pr", name="h2")
    nc.scalar.activation(h2, pe2, AF.Relu, bias=s["ub1c"][:, 0:1])
    s2 = prep.tile([128, TL], f32r, tag="pr", name="s2")
    nc.scalar.activation(s2, pt2, AF.Relu, bias=s["tb1c"][:, 0:1])
    yield
    pe3 = ppsum.tile([E, U], f32, tag="ps_o", name="pe3")
    _mm(nc, pe3, s["uw2T"], h2)
    pt3 = ppsum.tile([E, TL], f32, tag="ps_o", name="pt3")
    _mm(nc, pt3, s["tw2T"], s2)
    yield
    def _ident(out, in_, bias_ap):
        if CFG["pid"]:
            nc.vector.tensor_scalar_add(out, in_, bias_ap)
        else:
            nc.scalar.activation(out, in_, AF.Identity, bias=bias_ap)

    for h in range(2):
        _ident(uqT_s[:, h * U : (h + 1) * U], pe3, s["hq2T"][:, h : h + 1])
    yield
    for h in range(2, 4):
        _ident(uqT_s[:, h * U : (h + 1) * U], pe3, s["hq2T"][:, h : h + 1])
    teT = prep.tile([E, TL], f32r, tag="pr", name="teT")
    _ident(teT, pt3, s["tb2c"][:, 0:1])
    yield
    for kt in range(NKT):
        pb = ppsum.tile([128, TL], f32, tag="ps_o", name=f"pb{kt}")
        _mm(nc, pb, s["WtT"][:, kt * 128 : (kt + 1) * 128], teT)
        nc.vector.tensor_copy(out=b_s[kt], in_=pb)
        yield
    for kt in range(NKT):
        for half in range(2):
            pa = ppsum.tile([128, 512], f32, tag="ps_o", name=f"pa{kt}{half}")
            _mm(nc, pa, s["WuT"][:, kt * 128 : (kt + 1) * 128],
                uqT_s[:, half * 512 : (half + 1) * 512])
            _ident(a16_s[kt][:, half * 512 : (half + 1) * 512], pa,
                   s["fb0c"][:, kt : kt + 1])
            yield


def _emit_fusion(nc, d, s, pools, st, prep_gen, it=0):
    singles, prep, ppsum, rpool, opool, fpsum = pools
    a16_s, b_s = st["a16"], st["b"]

    ps_all = fpsum.tile([128, HU], f32, tag="ps_f", name=f"ps_all{it}")
    for r in range(NROUNDS):
        asg = _assign_units(r)
        rt = {}
        for (kt, j) in CONS_ORDER:
            t = 4 * r + j
            Rt = rpool.tile([128, HU], f16, tag="R",
                            name=f"R{it}_{r}_{j}_{kt}")
            bias_ap = b_s[kt][:, t : t + 1]
            eng = asg[(kt, j)]
            if eng == "act":
                nc.scalar.activation(Rt, a16_s[kt], AF.Relu, bias=bias_ap)
            elif eng == "pool":
                nc.gpsimd.tensor_scalar(
                    out=Rt, in0=a16_s[kt], scalar1=bias_ap,
                    scalar2=0.0, op0=ALU.add, op1=ALU.max,
                )
            else:
                nc.vector.tensor_scalar(
                    out=Rt, in0=a16_s[kt], scalar1=bias_ap,
                    scalar2=0.0, op0=ALU.add, op1=ALU.max,
                )
            rt[(kt, j)] = Rt
        for _ in range(CFG["ppk"]):
            next(prep_gen, None)
        mw = CFG["mw"]
        for kt in range(NKT):
            for half in range(2):
                for j in (1, 2, 3, 0):
                    nc.tensor.matmul(
                        ps_all[32 * j : 32 * j + mw,
                               half * 512 : (half + 1) * 512],
                        s["fw1z"][:, (kt * NROUNDS + r) * 32 :
                                  (kt * NROUNDS + r) * 32 + mw],
                        rt[(kt, j)][:, half * 512 : (half + 1) * 512],
                        start=(r == 0 and kt == 0),
                        stop=(r == NROUNDS - 1 and kt == NKT - 1),
                        tile_position=(0, 32 * j),
                        skip_group_check=True,
                    )
    for _ in prep_gen:
        pass

    # ---- single eviction (+fb1), then one gather DMA per quad ----
    # psum partition 32j + r holds t = 4r + j; each DMA j covers rows
    # t = j, j+4, ..., j+60 (dram stride 4) from partitions 32j..32j+15.
    o_st = opool.tile([128, HU], f32, tag="o", name=f"o{it}")
    odst = d["out"].rearrange("(r j) n -> r j n", j=4)
    for half in range(2):
        cs = slice(half * 512, (half + 1) * 512)
        if CFG["pev"]:
            nc.vector.tensor_scalar_add(o_st[:, cs], ps_all[:, cs],
                                        s["fb1s"][:, 0:1])
        else:
            nc.scalar.activation(o_st[:, cs], ps_all[:, cs], AF.Identity,
                                 bias=s["fb1s"][:, 0:1])
        for j in range(4):
            nc.sync.dma_start(out=odst[:, j, cs],
                              in_=o_st[32 * j : 32 * j + NROUNDS, cs])


def _build_nc(mult=1, loop=None):
    nc = bacc.Bacc(None, target_bir_lowering=False)
    d = {}
    for name, shape, dt_ in IN_SPECS:
        d[name] = nc.dram_tensor(name, list(shape), dt_, kind="ExternalInput")
    d["out"] = nc.dram_tensor("out", [TL, HU], f32, kind="ExternalOutput")

    pp = CFG["pp"] and loop is not None
    with TileContext(nc) as tc:
        with tc.tile_pool(name="singles", bufs=1) as singles, \
             tc.tile_pool(name="prep", bufs=2) as prep, \
             tc.tile_pool(name="rpool", bufs=CFG["rpool"]) as rpool, \
             tc.tile_pool(name="opool", bufs=2) as opool, \
             tc.tile_pool(name="fpsum", bufs=2, space="PSUM") as fpsum:
            pools = (singles, prep, fpsum, rpool, opool, fpsum)
            s = _emit_loads(nc, d, singles)
            hints = (ET.PE, ET.Activation, ET.DVE) + (
                (ET.Pool,) if CFG["xg"] else ())
            if pp:
                st = [_make_state(singles, p) for p in range(2)]
                for _ in _emit_prep(nc, s, pools, st[0]):
                    pass
                with tc.For_i(0, loop // 2, 1, hint_engines=hints):
                    for p in range(2):
                        _emit_fusion(nc, d, s, pools, st[p],
                                     _emit_prep(nc, s, pools, st[1 - p]),
                                     it=p)
            else:
                st = _make_state(singles, 0)
                ctx = (tc.For_i(0, loop, 1, hint_engines=hints)
                       if loop else contextlib.nullcontext())
                with ctx:
                    for it in range(mult):
                        for _ in _emit_prep(nc, s, pools, st):
                            pass
                        _emit_fusion(nc, d, s, pools, st, iter(()), it=it)

    nc.finalize()
    return nc


_NC_CACHE = {}


def _get_nc(mult=1, loop=None):
    key = (mult, loop, tuple(sorted(CFG.items())))
    if key not in _NC_CACHE:
        _NC_CACHE[key] = _build_nc(mult, loop)
    return _NC_CACHE[key]


def _prep_inputs(inputs):
    ct = np.ascontiguousarray
    f = np.float32
    uav_feat = inputs["uav_feat"].astype(f)
    task_feat = inputs["task_feat"].astype(f)
    fw1_kt = inputs["fw1"].reshape(NKT, 128).astype(np.float16)
    fw1z = np.zeros((128, NKT * NROUNDS * 32), dtype=np.float16)
    for kt in range(NKT):
        for r in range(NROUNDS):
            fw1z[:, (kt * NROUNDS + r) * 32 + r] = fw1_kt[kt]
    base = {
        "uavT": ct(uav_feat.T),
        "uw0T": ct(inputs["uw0"].T.astype(f)),
        "uw1T": ct(inputs["uw1"].T.astype(f)),
        "uw2T": ct(inputs["uw2"].T.astype(f)),
        "ub0c": ct(inputs["ub0"].astype(f).reshape(128, 1)),
        "ub1c": ct(inputs["ub1"].astype(f).reshape(128, 1)),
        "hq2T": ct((inputs["head_queries"].astype(f)
                    + inputs["ub2"].astype(f)[None, :]).T),
        "tw0T": ct(inputs["tw0"].T.astype(f)),
        "tw1T": ct(inputs["tw1"].T.astype(f)),
        "tw2T": ct(inputs["tw2"].T.astype(f)),
        "tb0c": ct(inputs["tb0"].astype(f).reshape(128, 1)),
        "tb1c": ct(inputs["tb1"].astype(f).reshape(128, 1)),
        "tb2c": ct(inputs["tb2"].astype(f).reshape(128, 1)),
        "WuT": ct(inputs["fw0"][:, :E].T.astype(f)),
        "WtT": ct(inputs["fw0"][:, E:].T.astype(f)),
        "fb0c": ct(inputs["fb0"].astype(f).reshape(NKT, 128).T),
        "fw1z": fw1z,
        "fb1s": ct(np.full((128, 1), float(inputs["fb1"][0]), dtype=f)),
    }
    taskT_full = ct(task_feat.T)
    in_maps = []
    for c in range(NCORES):
        m = dict(base)
        m["taskT"] = ct(taskT_full[:, c * TL : (c + 1) * TL])
        in_maps.append(m)
    return in_maps


def run(trace=False, **inputs):
    nc = _get_nc()
    in_maps = _prep_inputs(inputs)
    res = run_bass_kernel_spmd(nc, in_maps, list(range(NCORES)), trace=trace)
    big = np.concatenate([res.results[c]["out"] for c in range(NCORES)], axis=0)
    out = np.ascontiguousarray(big.T).reshape(H, U, T)
    return out, res


def kernel(**inputs):
    out, _ = run(**inputs)
    return out
